# revision 29
# baseline (speedup 1.0000x reference)
"""BiologicalGAT forward on 8 Trainium2 NeuronCores.

Strategy (dst-sharded, all-gather of projected features):
  - Nodes are sharded contiguously across 8 cores (1250 each). Edges
    (incl. self-loops) are sorted by destination and assigned to the core
    owning the destination node, grouped into 128-destination blocks.
  - Per layer: each core computes h_ext = x_shard @ [W | W@As | W@Ad]
    (projected features + per-node attention scores) for its shard, then an
    AllGather replicates the full h_ext table. Per destination block the
    core dma_gathers the h_ext rows of the edge sources (the memory-bound
    part), computes edge softmax weights on-chip, and reduces messages with
    a selector matmul into PSUM. Softmax normalization happens after the
    segment sums (exp-max subtraction is skipped: scores are O(0.1) so
    exp never overflows and alpha is mathematically identical).
  - LayerNorm / ReLU / residual run on the owned 1250 rows only. The
    global mean pool is a per-core partial matmul + a tiny AllReduce.

The program is specialized at kernel() time to the actual edge structure
(per-block chunk counts are compile-time constants, padded to the max
across cores so one SPMD program serves all 8 cores).
"""

import contextlib
import ctypes
import math
import os
import sys
import types

import numpy as np

# ---------------------------------------------------------------- axon setup

_SO_PATH = "/opt/axon/libaxon_pjrt.so"


def _ntff_profile_via_ctypes(so_path):
    lib = ctypes.CDLL(so_path)
    if not hasattr(lib, "axon_start_nrt_profile"):
        return None
    lib.axon_start_nrt_profile.argtypes = [
        ctypes.POINTER(ctypes.c_int64),
        ctypes.c_size_t,
    ]
    lib.axon_start_nrt_profile.restype = ctypes.c_int64
    lib.axon_stop_nrt_profile.argtypes = [ctypes.c_char_p]
    lib.axon_stop_nrt_profile.restype = ctypes.c_int64

    @contextlib.contextmanager
    def _hook(output_dir, device_ids):
        import jax

        jax.devices()
        if device_ids:
            ids = (ctypes.c_int64 * len(device_ids))(*device_ids)
            rc = lib.axon_start_nrt_profile(ids, len(device_ids))
        else:
            rc = lib.axon_start_nrt_profile(None, 0)
        if rc != 0:
            raise RuntimeError(f"axon_start_nrt_profile rc={rc}")
        try:
            yield
        finally:
            n = lib.axon_stop_nrt_profile(str(output_dir).encode())
            if n < 0:
                raise RuntimeError(f"axon_stop_nrt_profile rc={n}")

    return _hook


def _install_axon_hooks():
    if "antenv.axon_hooks" in sys.modules:
        return
    mod = types.ModuleType("antenv.axon_hooks")
    holder = [None]
    mod.set_axon_ntff_profile_hook = lambda h: holder.__setitem__(0, h)
    mod.get_axon_ntff_profile_hook = lambda: holder[0]
    sys.modules["antenv.axon_hooks"] = mod
    try:
        import antenv

        antenv.axon_hooks = mod
    except ImportError:
        pass
    if os.path.exists(_SO_PATH):
        mod.set_axon_ntff_profile_hook(_ntff_profile_via_ctypes(_SO_PATH))


_install_axon_hooks()

import concourse.bacc as bacc  # noqa: E402
import concourse.bass as bass  # noqa: E402
import concourse.mybir as mybir  # noqa: E402
import concourse.tile as tile  # noqa: E402
from concourse._compat import get_trn_type  # noqa: E402
from concourse.bass_utils import run_bass_kernel_spmd  # noqa: E402
from concourse.library_config import mlp as _mlp_lib  # noqa: E402

try:
    from concourse import bass_utils as _bu

    _bu.upload_artifacts = lambda tmpdir: tmpdir
except Exception:
    pass

# ---------------------------------------------------------------- wait legalization

_waitfix_counter = [0]


def _legalize_waits(nc):
    """walrus allows 1 sem wait on most instructions, 2 on EventSemaphore.
    Hoist excess waits onto standalone event-sem carriers on the same engine
    (the sequencer blocks on the carrier first; program order makes it
    equivalent)."""
    added = 0
    for bb in nc.main_func.blocks:
        insts = bb.instructions
        i = 0
        while i < len(insts):
            inst = insts[i]
            si = inst.sync_info
            if si is None or not si.on_wait:
                i += 1
                continue
            lim = 2 if type(inst).__name__ == "InstEventSemaphore" else 1
            waits = list(si.on_wait)
            if len(waits) <= lim:
                i += 1
                continue
            keep, excess = waits[:lim], waits[lim:]
            carriers = []
            for j in range(0, len(excess), 2):
                _waitfix_counter[0] += 1
                c = mybir.InstEventSemaphore(
                    name=f"waitfix_{_waitfix_counter[0]}", ins=[], outs=[]
                )
                c.engine = inst.engine
                c.sync_info = mybir.SyncInfo(on_wait=excess[j : j + 2], on_update=[])
                carriers.append(c)
            inst.sync_info = mybir.SyncInfo(on_wait=keep, on_update=list(si.on_update))
            for k, c in enumerate(carriers):
                insts.insert(i + k, c)
                nc.register_instruction(c, overwrite=True)
            added += len(carriers)
            i += len(carriers) + 1
    return added


# ---------------------------------------------------------------- constants

N_NODES = 10000
N_EDGES = 160000
HID = 512
HEADS = 4
N_GRAPHS = 8
NCORES = 8
SHARD = N_NODES // NCORES  # 1250
P = 128
NBLK = math.ceil(SHARD / P)  # 10
TW = 576  # h_ext table row width (f32); 576*4 % 256 == 0 for dma_gather
GSUB = 4  # chunks per dma_gather call (512 indices)
EPS_LN = 1e-5
LAYERS = [(4, 128), (4, 128), (1, 512)]  # (heads, ch/head)
F32 = mybir.dt.float32
F32R = mybir.dt.float32r
I16 = mybir.dt.int16


def _r(ap):
    return ap.bitcast(F32R)

_cache = {}


# ---------------------------------------------------------------- host prep


def _host_prep(edge_index, node_types, batch):
    src = np.concatenate([edge_index[0], np.arange(N_NODES, dtype=np.int64)])
    dst = np.concatenate([edge_index[1], np.arange(N_NODES, dtype=np.int64)])
    order = np.argsort(dst, kind="stable")
    src, dst = src[order], dst[order]
    # edges per (core, block): dst block = dst // 128 within shard
    # block boundaries in sorted dst via searchsorted
    starts = np.searchsorted(dst, np.arange(0, N_NODES + 1, P))  # 79 blocks of 128
    # but shard blocks: core c block b covers nodes [c*1250 + b*128, ...)
    block_lo = []
    block_hi = []
    for c in range(NCORES):
        base = c * SHARD
        for b in range(NBLK):
            lo = base + b * P
            hi = min(base + (b + 1) * P, base + SHARD)
            block_lo.append(lo)
            block_hi.append(hi)
    lo_idx = np.searchsorted(dst, np.array(block_lo))
    hi_idx = np.searchsorted(dst, np.array(block_hi))
    counts = (hi_idx - lo_idx).reshape(NCORES, NBLK)
    cpb = np.maximum(1, np.ceil(counts.max(axis=0) / P).astype(np.int64))  # [NBLK]
    totc = int(cpb.sum())

    # per-core tensors
    per_core = []
    # sub-gather segmentation (same for all cores): per block, chunks split
    # into calls of <= GSUB chunks
    subs = []  # list of (block, k0, kn, goff) with goff = global chunk offset
    goffs = np.concatenate([[0], np.cumsum(cpb)])
    for b in range(NBLK):
        k0 = 0
        while k0 < cpb[b]:
            kn = min(GSUB, int(cpb[b]) - k0)
            subs.append((b, k0, kn, int(goffs[b]) + k0))
            k0 += kn

    for c in range(NCORES):
        idx16 = np.zeros((16, totc * 8), np.int16)
        dstloc = np.full((P, totc), 255, np.uint8)
        dstrep = np.full((totc * P,), 255, np.uint8)
        for b in range(NBLK):
            i0, i1 = lo_idx[c * NBLK + b], hi_idx[c * NBLK + b]
            nsl = int(cpb[b]) * P
            s_pad = np.zeros(nsl, np.int64)
            d_pad = np.full(nsl, -1.0, np.float32)
            s_pad[: i1 - i0] = src[i0:i1]
            d_pad[: i1 - i0] = (dst[i0:i1] - block_lo[c * NBLK + b]).astype(np.float32)
            goff = int(goffs[b])
            dl = np.full((nsl,), 255, np.uint8)
            m2 = d_pad >= 0
            dl[m2] = d_pad[m2].astype(np.uint8)
            dstloc[:, goff : goff + int(cpb[b])] = dl.reshape(int(cpb[b]), P).T
            dmask = d_pad >= 0
            dstrep[goff * P : goff * P + nsl][dmask] = d_pad[dmask].astype(np.uint8)
            # idx16 layout per sub-gather: arr[kn,128] -> [16, kn*8]
            for bb2, k0, kn, go in subs:
                if bb2 != b:
                    continue
                arr = s_pad[k0 * P : (k0 + kn) * P].reshape(kn, 8, 16)
                lay = arr.transpose(2, 0, 1).reshape(16, kn * 8)
                idx16[:, go * 8 : (go + kn) * 8] = lay.astype(np.int16)
        idx16 = np.tile(idx16, (8, 1))

        base = c * SHARD
        ohb = np.zeros((P, NBLK * N_GRAPHS), np.float32)
        for b in range(NBLK):
            nb = min(P, SHARD - b * P)
            rows = batch[base + b * P : base + b * P + nb].astype(np.int64)
            ohb[np.arange(nb), b * N_GRAPHS + rows] = 1.0
        oh6 = np.zeros((6, SHARD), np.float32)
        oh6[node_types[base : base + SHARD].astype(np.int64), np.arange(SHARD)] = 1.0
        per_core.append(
            {
                "idx16": idx16,
                "dstloc": dstloc,
                "ohb": ohb,
                "oh6T": oh6,
                "dstrep": np.tile(dstrep[None, :], (P, 1)),
            }
        )
    meta = {
        "cpb": tuple(int(x) for x in cpb),
        "totc": totc,
        "subs": tuple(subs),
    }
    return meta, per_core


def _build_wext(W, a_src, a_dst, H, C):
    As = np.zeros((HID, H), np.float32)
    Ad = np.zeros((HID, H), np.float32)
    for h in range(H):
        As[h * C : (h + 1) * C, h] = a_src[h]
        Ad[h * C : (h + 1) * C, h] = a_dst[h]
    out = np.zeros((HID, TW), np.float32)
    out[:, :HID] = W
    out[:, HID : HID + H] = W @ As
    out[:, HID + H : HID + 2 * H] = W @ Ad
    return out


# ---------------------------------------------------------------- program


def _build_program(meta):
    cpb = meta["cpb"]
    totc = meta["totc"]
    subs = meta["subs"]
    cpbmax_sub = GSUB

    nc = bacc.Bacc(get_trn_type() or "TRN2")
    dt = F32

    # I/O
    xT_in = nc.dram_tensor("xT", [HID, SHARD], F32R, kind="ExternalInput")
    oh6T_in = nc.dram_tensor("oh6T", [6, SHARD], F32R, kind="ExternalInput")
    temb_in = nc.dram_tensor("temb", [6, HID], F32R, kind="ExternalInput")
    wext_in = nc.dram_tensor("wext", [3, HID, TW], F32R, kind="ExternalInput")
    brep_in = nc.dram_tensor("brep", [3, P, HID], dt, kind="ExternalInput")
    grep_in = nc.dram_tensor("grep", [3, P, HID], dt, kind="ExternalInput")
    lbrep_in = nc.dram_tensor("lbrep", [3, P, HID], dt, kind="ExternalInput")
    colidx_in = nc.dram_tensor("colidx", [P, P], mybir.dt.uint8, kind="ExternalInput")
    ident_in = nc.dram_tensor("ident", [P, P], dt, kind="ExternalInput")
    dstloc_in = nc.dram_tensor("dstloc", [P, totc], mybir.dt.uint8, kind="ExternalInput")
    dstrep_in = nc.dram_tensor("dstrep", [P, totc * P], mybir.dt.uint8, kind="ExternalInput")
    iotap_in = nc.dram_tensor("iotap", [P, 1], mybir.dt.uint8, kind="ExternalInput")
    idx16_in = nc.dram_tensor("idx16", [P, totc * 8], I16, kind="ExternalInput")
    ohb_in = nc.dram_tensor("ohb", [P, NBLK * N_GRAPHS], dt, kind="ExternalInput")
    invc_in = nc.dram_tensor("invc", [N_GRAPHS, 1], dt, kind="ExternalInput")
    out_nodes = nc.dram_tensor("out_nodes", [SHARD, HID], dt, kind="ExternalOutput")
    out_graph = nc.dram_tensor("out_graph", [N_GRAPHS, HID], dt, kind="ExternalOutput")

    # internal dram
    h_shard_d = nc.dram_tensor("h_shard_d", [SHARD, TW], F32R)
    h_full_d = nc.dram_tensor("h_full_d", [N_NODES, TW], F32R, addr_space="Shared")
    pool_in_d = nc.dram_tensor("pool_in_d", [N_GRAPHS, HID], dt)
    pool_out_d = nc.dram_tensor("pool_out_d", [N_GRAPHS, HID], dt, addr_space="Shared")

    nc.gpsimd.load_library(_mlp_lib)

    with tile.TileContext(nc) as tc:
        with (
            tc.tile_pool(name="const", bufs=1) as cst,
            tc.tile_pool(name="xt", bufs=1) as xtp,
            tc.tile_pool(name="xblk", bufs=1) as xbp,
            tc.tile_pool(name="wextp", bufs=1) as wxp,
            tc.tile_pool(name="g", bufs=4) as gp,
            tc.tile_pool(name="sel", bufs=2) as selp,
            tc.tile_pool(name="selt", bufs=2) as seltp,
            tc.tile_pool(name="wt", bufs=2) as wtp,
            tc.tile_pool(name="hstage", bufs=2) as hsp,
            tc.tile_pool(name="hz", bufs=3) as hzp,
            tc.tile_pool(name="pbig", bufs=2, space="PSUM") as pbig,
            tc.tile_pool(name="pselT", bufs=2, space="PSUM") as pselT,
            tc.tile_pool(name="ptiny", bufs=2, space="PSUM") as ptiny,
            tc.tile_pool(name="psacc", bufs=1, space="PSUM") as psacc,
            tc.tile_pool(name="ppool", bufs=1, space="PSUM") as ppoolp,
        ):
            V, A = nc.vector, nc.scalar
            Copy = mybir.ActivationFunctionType.Copy
            Ident = mybir.ActivationFunctionType.Identity
            Exp = mybir.ActivationFunctionType.Exp
            Ln = mybir.ActivationFunctionType.Ln
            Square = mybir.ActivationFunctionType.Square
            Relu = mybir.ActivationFunctionType.Relu

            # ---- consts
            colidx = cst.tile([P, P], mybir.dt.uint8)
            nc.sync.dma_start(colidx[:], colidx_in[:])
            ident = cst.tile([P, P], dt)
            nc.sync.dma_start(ident[:], ident_in[:])
            dstloc = cst.tile([P, totc], mybir.dt.uint8)
            nc.sync.dma_start(dstloc[:], dstloc_in[:])
            dstrep = cst.tile([P, totc * P], mybir.dt.uint8)
            nc.sync.dma_start(dstrep[:], dstrep_in[:])
            iota_u8 = cst.tile([P, 1], mybir.dt.uint8, tag="iota_u8")
            nc.sync.dma_start(iota_u8[:], iotap_in[:])
            idx16 = cst.tile([P, totc * 8], I16)
            nc.sync.dma_start(idx16[:], idx16_in[:])
            temb = cst.tile([6, HID], F32R)
            nc.sync.dma_start(temb[:], temb_in[:])
            oh6T = cst.tile([6, SHARD], F32R)
            nc.sync.dma_start(oh6T[:], oh6T_in[:])
            ohb = cst.tile([P, NBLK * N_GRAPHS], dt)
            nc.sync.dma_start(ohb[:], ohb_in[:])
            invc = cst.tile([N_GRAPHS, 1], dt)
            nc.sync.dma_start(invc[:], invc_in[:])
            breps, greps, lbreps = [], [], []
            for l in range(3):
                t = cst.tile([P, HID], dt, tag=f"brep{l}")
                nc.sync.dma_start(t[:], brep_in[l])
                breps.append(t)
                t = cst.tile([P, HID], dt, tag=f"grep{l}")
                nc.sync.dma_start(t[:], grep_in[l])
                greps.append(t)
                t = cst.tile([P, HID], dt, tag=f"lbrep{l}")
                nc.sync.dma_start(t[:], lbrep_in[l])
                lbreps.append(t)
            alphad = cst.tile([P, NBLK * 16], F32R)
            V.memset(alphad[:].bitcast(F32), 0.0)
            c_magic = cst.tile([P, 1], mybir.dt.int32, tag="c_magic")
            V.memset(c_magic[:], 0x5F3759DF)

            xt_a = xtp.tile([P, 4, SHARD], F32R, tag="xt_a")
            xt_b = xtp.tile([P, 4, SHARD], F32R, tag="xt_b")
            xblk_a = xbp.tile([P, NBLK, HID], dt, tag="xblk_a")
            xblk_b = xbp.tile([P, NBLK, HID], dt, tag="xblk_b")

            # ---- x0T = xT + type_emb.T @ oh6T
            nc.sync.dma_start(
                xt_a[:], xT_in[:].rearrange("(k p) n -> p k n", p=P)
            )
            ntiles = [(0, 512), (512, 512), (1024, SHARD - 1024)]
            for kc in range(4):
                for n0, nn in ntiles:
                    pt = pbig.tile([P, 512], dt, tag="big")
                    nc.tensor.matmul(
                        pt[:, :nn],
                        lhsT=temb[:, kc * P : (kc + 1) * P],
                        rhs=oh6T[:, n0 : n0 + nn],
                        start=True,
                        stop=True,
                    )
                    V.tensor_add(
                        xt_a[:, kc, n0 : n0 + nn],
                        xt_a[:, kc, n0 : n0 + nn],
                        pt[:, :nn],
                    )

            goffs = [0]
            for b in range(NBLK):
                goffs.append(goffs[-1] + cpb[b])

            ppool = ppoolp.tile([N_GRAPHS, HID], dt)

            xts = [xt_a, xt_b, xt_a]
            xbs = [xblk_a, xblk_b, xblk_a]  # output blocks per layer

            def emit_dense_block(lx, m, wx, xt_src):
                Hx = LAYERS[lx][0]
                nm = min(P, SHARD - m * P)
                ph = pbig.tile([P, 512], dt, tag="big")
                pa = ptiny.tile([P, 16 * GSUB], dt, tag="tiny")
                for kc in range(4):
                    lslice = xt_src[:, kc, m * P : m * P + nm]
                    nc.tensor.matmul(
                        ph[:nm, :],
                        lhsT=lslice,
                        rhs=wx[:, kc, 0:HID],
                        start=(kc == 0),
                        stop=(kc == 3),
                    )
                    nc.tensor.matmul(
                        pa[:nm, : 2 * Hx],
                        lhsT=lslice.bitcast(F32),
                        rhs=wx[:, kc, HID : HID + 2 * Hx].bitcast(F32),
                        start=(kc == 0),
                        stop=(kc == 3),
                    )
                hx = hsp.tile([P, TW], F32R, tag="hx")
                A.activation(hx[:nm, :HID], ph[:nm, :], Copy)
                A.activation(hx[:nm, HID : HID + 2 * Hx], pa[:nm, : 2 * Hx], Copy)
                V.tensor_copy(
                    alphad[:nm, m * 16 : m * 16 + Hx], pa[:nm, Hx : 2 * Hx]
                )
                nc.sync.dma_start(h_shard_d[m * P : m * P + nm, :], hx[:nm, :])

            def emit_ag():
                nc.gpsimd.collective_compute(
                    "AllGather",
                    mybir.AluOpType.bypass,
                    ins=[h_shard_d[:]],
                    outs=[h_full_d[:]],
                    replica_groups=[list(range(NCORES))],
                )

            # layer-0 dense up front
            wx = wxp.tile([P, 4, TW], F32R, tag="wext")
            nc.sync.dma_start(wx[:], wext_in[0].rearrange("(k p) w -> p k w", p=P))
            for m in range(NBLK):
                emit_dense_block(0, m, wx, xts[0])
            emit_ag()

            for l in range(3):
                H, C = LAYERS[l]
                xt_cur = xts[l]
                xb_out = xbs[l]
                xb_res = xbs[l - 1] if l >= 1 else None
                xt_nxt = xts[l + 1] if l < 2 else None

                # ---- gather phase
                for b in range(NBLK):
                    nb = min(P, SHARD - b * P)
                    pm = pbig.tile([P, 512], dt, tag="big")
                    ps = psacc.tile([P, 16], dt)
                    nchunks = cpb[b]
                    bsubs = [s for s in subs if s[0] == b]
                    for (bb, k0, kn, go) in bsubs:
                        G = gp.tile([P, cpbmax_sub, TW], F32R, tag="g")
                        nc.gpsimd.dma_gather(
                            G[:, :kn, :],
                            h_full_d[:],
                            idx16[:, go * 8 : (go + kn) * 8],
                            kn * P,
                            kn * P,
                            TW,
                            single_packet=False,
                        )
                        # group-batched selectors
                        sel8 = selp.tile([P, cpbmax_sub, P], F32R, tag="sel")
                        V.tensor_tensor(
                            out=sel8[:, :kn, :],
                            in0=dstloc[:, go : go + kn].to_broadcast([P, kn, P]),
                            in1=colidx[:]
                            .rearrange("p (a c) -> p a c", a=1)
                            .to_broadcast([P, kn, P]),
                            op=mybir.AluOpType.is_equal,
                        )
                        selT8 = seltp.tile([P, cpbmax_sub, P], F32R, tag="selT")
                        V.tensor_tensor(
                            out=selT8[:, :kn, :],
                            in0=iota_u8[:, 0:1].to_broadcast([P, kn, P]),
                            in1=dstrep[:, go * P : (go + kn) * P].rearrange(
                                "p (a c) -> p a c", c=P
                            ),
                            op=mybir.AluOpType.is_equal,
                        )
                        # alpha_dst per edge: kn small matmuls into one psum bank
                        pad8 = ptiny.tile([P, 16 * cpbmax_sub], dt, tag="tiny")
                        for kk in range(kn):
                            nc.tensor.matmul(
                                pad8[:, kk * 16 : kk * 16 + 16],
                                lhsT=selT8[:, kk, :],
                                rhs=alphad[:, b * 16 : b * 16 + 16],
                                start=True,
                                stop=True,
                            )
                        # batched scores: es = alpha_src + alpha_dst; w = exp(leaky(es))
                        es = wtp.tile([P, 4 * cpbmax_sub], dt, tag="es")
                        V.tensor_tensor(
                            out=es[:, : kn * H].rearrange("p (a c) -> p a c", c=H),
                            in0=G[:, :kn, HID : HID + H],
                            in1=pad8[:, : kn * 16]
                            .rearrange("p (a c) -> p a c", c=16)[:, :, :H],
                            op=mybir.AluOpType.add,
                        )
                        lk = wtp.tile([P, 4 * cpbmax_sub], dt, tag="lk")
                        V.tensor_scalar(
                            out=lk[:, : kn * H],
                            in0=es[:, : kn * H],
                            scalar1=0.2,
                            scalar2=None,
                            op0=mybir.AluOpType.mult,
                        )
                        V.tensor_tensor(
                            out=lk[:, : kn * H],
                            in0=es[:, : kn * H],
                            in1=lk[:, : kn * H],
                            op=mybir.AluOpType.max,
                        )
                        wv = wtp.tile([P, 4 * cpbmax_sub], dt, tag="wv")
                        A.activation(wv[:, : kn * H], lk[:, : kn * H], Exp)
                        # multiply w into G in place (rounding to f32r for PE),
                        # and stash w into the pad columns of each row
                        for kk in range(kn):
                            for h in range(H):
                                A.activation(
                                    G[:, kk, h * C : (h + 1) * C],
                                    G[:, kk, h * C : (h + 1) * C],
                                    Copy,
                                    scale=wv[:, kk * H + h : kk * H + h + 1],
                                )
                        V.tensor_copy(
                            G[:, :kn, HID + 2 * H : HID + 3 * H],
                            wv[:, : kn * H].rearrange("p (a h) -> p a h", h=H),
                        )
                        for kk in range(kn):
                            k = k0 + kk
                            nc.tensor.matmul(
                                pm[:],
                                lhsT=sel8[:, kk, :],
                                rhs=G[:, kk, :HID],
                                start=(k == 0),
                                stop=(k == nchunks - 1),
                            )
                            nc.tensor.matmul(
                                ps[:, :16],
                                lhsT=sel8[:, kk, :],
                                rhs=G[:, kk, HID + 2 * H : HID + 2 * H + 16],
                                start=(k == 0),
                                stop=(k == nchunks - 1),
                            )
                    # ---- block epilogue
                    sS = wtp.tile([P, 8], dt, tag="sS")
                    A.activation(sS[:, :H], ps[:, :H], Copy, bias=1e-16)
                    rs = wtp.tile([P, 8], dt, tag="rs")
                    V.reciprocal(rs[:, :H], sS[:, :H])
                    zt = hzp.tile([P, HID], dt, tag="hz")
                    z = zt
                    for h in range(H):
                        A.activation(
                            z[:, h * C : (h + 1) * C],
                            pm[:, h * C : (h + 1) * C],
                            Copy,
                            scale=rs[:, h : h + 1],
                        )
                    V.tensor_add(z[:, :HID], z[:, :HID], breps[l][:])
                    if l < 2:
                        z2 = hzp.tile([P, HID], dt, tag="hz")
                        A.activation(z2[:, :HID], z[:, :HID], Relu)
                        z = z2
                    # LayerNorm
                    sumz = wtp.tile([P, 8], dt, tag="sumz")
                    V.reduce_sum(sumz[:, 0:1], z[:, :HID], axis=mybir.AxisListType.X)
                    mu = wtp.tile([P, 8], dt, tag="mu")
                    V.tensor_scalar(
                        out=mu[:, 0:1], in0=sumz[:, 0:1], scalar1=1.0 / HID,
                        scalar2=None, op0=mybir.AluOpType.mult,
                    )
                    y0 = hzp.tile([P, HID], dt, tag="hz")
                    s2 = wtp.tile([P, 8], dt, tag="s2")
                    A.activation(y0[:, :HID], z[:, :HID], Square, accum_out=s2[:, 0:1])
                    stat = wtp.tile([P, 8], dt, tag="stat")
                    V.tensor_scalar(
                        out=stat[:, 0:1], in0=s2[:, 0:1], scalar1=1.0 / HID,
                        scalar2=None, op0=mybir.AluOpType.mult,
                    )
                    V.tensor_tensor(
                        out=stat[:, 1:2], in0=mu[:, 0:1], in1=mu[:, 0:1],
                        op=mybir.AluOpType.mult,
                    )
                    V.tensor_tensor(
                        out=stat[:, 2:3], in0=stat[:, 0:1], in1=stat[:, 1:2],
                        op=mybir.AluOpType.subtract,
                    )
                    V.tensor_scalar(
                        out=stat[:, 2:3], in0=stat[:, 2:3], scalar1=EPS_LN,
                        scalar2=None, op0=mybir.AluOpType.add,
                    )
                    # rstd = rsqrt(var) via bit-trick + 3 Newton steps (DVE only;
                    # ACT sqrt has poor precision and lives in another table set)
                    vi = stat[:, 2:3].bitcast(mybir.dt.int32)
                    si = stat[:, 7:8].bitcast(mybir.dt.int32)
                    V.tensor_scalar(
                        out=si, in0=vi, scalar1=1, scalar2=None,
                        op0=mybir.AluOpType.logical_shift_right,
                    )
                    V.tensor_tensor(
                        out=si, in0=c_magic[:, 0:1], in1=si,
                        op=mybir.AluOpType.subtract,
                    )
                    yn = stat[:, 7:8]
                    tn = stat[:, 3:4]
                    for _ in range(3):
                        V.tensor_tensor(out=tn, in0=yn, in1=yn, op=mybir.AluOpType.mult)
                        V.tensor_tensor(out=tn, in0=tn, in1=stat[:, 2:3], op=mybir.AluOpType.mult)
                        V.tensor_scalar(out=tn, in0=tn, scalar1=-0.5, scalar2=None, op0=mybir.AluOpType.mult)
                        V.tensor_scalar(out=tn, in0=tn, scalar1=1.5, scalar2=None, op0=mybir.AluOpType.add)
                        V.tensor_tensor(out=yn, in0=yn, in1=tn, op=mybir.AluOpType.mult)
                    V.tensor_copy(stat[:, 4:5], yn)
                    V.tensor_tensor(
                        out=stat[:, 5:6], in0=mu[:, 0:1], in1=stat[:, 4:5],
                        op=mybir.AluOpType.mult,
                    )
                    V.tensor_scalar(
                        out=stat[:, 6:7], in0=stat[:, 5:6], scalar1=-1.0,
                        scalar2=None, op0=mybir.AluOpType.mult,
                    )
                    A.activation(
                        y0[:, :HID], z[:, :HID], Ident, scale=stat[:, 4:5], bias=stat[:, 6:7]
                    )
                    V.tensor_tensor(
                        out=y0[:, :HID], in0=y0[:, :HID], in1=greps[l][:], op=mybir.AluOpType.mult
                    )
                    ob = xb_out[:, b, :]
                    V.tensor_tensor(
                        out=ob, in0=y0[:, :HID], in1=lbreps[l][:], op=mybir.AluOpType.add
                    )
                    if l >= 1:
                        V.tensor_tensor(
                            out=ob, in0=ob, in1=xb_res[:, b, :],
                            op=mybir.AluOpType.add,
                        )
                    if l < 2:
                        for kc in range(4):
                            pT2 = pselT.tile([P, P], dt, tag="selT")
                            nc.tensor.transpose(
                                pT2[:], xb_out[:, b, kc * P : (kc + 1) * P], ident[:]
                            )
                            A.activation(
                                xt_nxt[:, kc, b * P : b * P + nb],
                                pT2[:, :nb],
                                Copy,
                            )
                    else:
                        nc.sync.dma_start(
                            out_nodes[b * P : b * P + nb, :], ob[:nb]
                        )
                        nc.tensor.matmul(
                            ppool[:],
                            lhsT=ohb[:, b * N_GRAPHS : (b + 1) * N_GRAPHS],
                            rhs=ob,
                            start=(b == 0),
                            stop=(b == NBLK - 1),
                        )

                if l < 2:
                    wx = wxp.tile([P, 4, TW], F32R, tag="wext")
                    nc.sync.dma_start(
                        wx[:], wext_in[l + 1].rearrange("(k p) w -> p k w", p=P)
                    )
                    for m in range(NBLK):
                        emit_dense_block(l + 1, m, wx, xt_nxt)
                    emit_ag()

            # ---- graph pooling
            pc = cst.tile([N_GRAPHS, HID], dt, tag="poolc")
            A.activation(pc[:], ppool[:], Copy)
            nc.sync.dma_start(pool_in_d[:], pc[:])
            nc.gpsimd.collective_compute(
                "AllReduce",
                mybir.AluOpType.add,
                ins=[pool_in_d[:]],
                outs=[pool_out_d[:]],
                replica_groups=[list(range(NCORES))],
            )
            pg = cst.tile([N_GRAPHS, HID], dt, tag="poolg")
            nc.sync.dma_start(pg[:], pool_out_d[:])
            V.tensor_scalar(
                out=pg[:], in0=pg[:], scalar1=invc[:, 0:1], scalar2=None,
                op0=mybir.AluOpType.mult,
            )
            nc.sync.dma_start(out_graph[:], pg[:])

    nc.compile()
    _legalize_waits(nc)
    return nc


# ---------------------------------------------------------------- runner


def _prepare(inputs):
    edge_index = np.asarray(inputs["edge_index"])
    node_types = np.asarray(inputs["node_types"])
    batch = np.asarray(inputs["batch"])
    meta, per_core = _host_prep(edge_index, node_types, batch)

    key = (meta["cpb"], meta["totc"])
    if key not in _cache:
        _cache.clear()
        _cache[key] = _build_program(meta)
    nc = _cache[key]

    x = np.ascontiguousarray(np.asarray(inputs["x"], np.float32))
    wexts = np.stack(
        [
            _build_wext(
                np.asarray(inputs[f"gat{l}_W"], np.float32),
                np.asarray(inputs[f"gat{l}_asrc"], np.float32),
                np.asarray(inputs[f"gat{l}_adst"], np.float32),
                *LAYERS[l],
            )
            for l in range(3)
        ]
    )
    brep = np.stack(
        [np.tile(np.asarray(inputs[f"gat{l}_b"], np.float32), (P, 1)) for l in range(3)]
    )
    ln_g = np.asarray(inputs["ln_g"], np.float32)
    ln_b = np.asarray(inputs["ln_b"], np.float32)
    grep = np.stack([np.tile(ln_g[l], (P, 1)) for l in range(3)])
    lbrep = np.stack([np.tile(ln_b[l], (P, 1)) for l in range(3)])
    colidx = np.tile(np.arange(P, dtype=np.uint8), (P, 1))
    ident = np.eye(P, dtype=np.float32)
    cnt = np.bincount(batch.astype(np.int64), minlength=N_GRAPHS).astype(np.float32)
    invc = (1.0 / np.maximum(cnt, 1.0)).reshape(N_GRAPHS, 1)

    in_maps = []
    for c in range(NCORES):
        base = c * SHARD
        pc = per_core[c]
        in_maps.append(
            {
                "xT": np.ascontiguousarray(x[base : base + SHARD].T),
                "oh6T": pc["oh6T"],
                "temb": np.asarray(inputs["type_emb"], np.float32),
                "wext": wexts,
                "brep": brep,
                "grep": grep,
                "lbrep": lbrep,
                "colidx": colidx,
                "ident": ident,
                "dstloc": pc["dstloc"],
                "dstrep": pc["dstrep"],
                "iotap": np.arange(P, dtype=np.uint8).reshape(P, 1),
                "idx16": pc["idx16"],
                "ohb": pc["ohb"],
                "invc": invc,
            }
        )
    return nc, in_maps


def _run(inputs, trace=False, tmpdir=None):
    nc, in_maps = _prepare(inputs)
    res = run_bass_kernel_spmd(
        nc, in_maps, list(range(NCORES)), trace=trace, tmpdir=tmpdir
    )
    node_emb = np.concatenate(
        [res.results[c]["out_nodes"] for c in range(NCORES)], axis=0
    )
    graph_emb = res.results[0]["out_graph"]
    return (node_emb, graph_emb), res


def kernel(**inputs):
    out, _ = _run(inputs)
    return out


# revision 32
# speedup vs baseline: 1.6718x; 1.6718x over previous
"""BiologicalGAT forward on 8 Trainium2 NeuronCores.

Strategy (dst-sharded, all-gather of projected features):
  - Nodes are sharded contiguously across 8 cores (1250 each). Edges
    (incl. self-loops) are sorted by destination and assigned to the core
    owning the destination node, grouped into 128-destination blocks.
  - Per layer: each core computes h_ext = x_shard @ [W | W@As | W@Ad]
    (projected features + per-node attention scores) for its shard, then an
    AllGather replicates the full h_ext table. Per destination block the
    core dma_gathers the h_ext rows of the edge sources (the memory-bound
    part), computes edge softmax weights on-chip, and reduces messages with
    a selector matmul into PSUM. Softmax normalization happens after the
    segment sums (exp-max subtraction is skipped: scores are O(0.1) so
    exp never overflows and alpha is mathematically identical).
  - LayerNorm / ReLU / residual run on the owned 1250 rows only. The
    global mean pool is a per-core partial matmul + a tiny AllReduce.

The program is specialized at kernel() time to the actual edge structure
(per-block chunk counts are compile-time constants, padded to the max
across cores so one SPMD program serves all 8 cores).
"""

import contextlib
import ctypes
import math
import os
import sys
import types

import numpy as np

# ---------------------------------------------------------------- axon setup

_SO_PATH = "/opt/axon/libaxon_pjrt.so"


def _ntff_profile_via_ctypes(so_path):
    lib = ctypes.CDLL(so_path)
    if not hasattr(lib, "axon_start_nrt_profile"):
        return None
    lib.axon_start_nrt_profile.argtypes = [
        ctypes.POINTER(ctypes.c_int64),
        ctypes.c_size_t,
    ]
    lib.axon_start_nrt_profile.restype = ctypes.c_int64
    lib.axon_stop_nrt_profile.argtypes = [ctypes.c_char_p]
    lib.axon_stop_nrt_profile.restype = ctypes.c_int64

    @contextlib.contextmanager
    def _hook(output_dir, device_ids):
        import jax

        jax.devices()
        if device_ids:
            ids = (ctypes.c_int64 * len(device_ids))(*device_ids)
            rc = lib.axon_start_nrt_profile(ids, len(device_ids))
        else:
            rc = lib.axon_start_nrt_profile(None, 0)
        if rc != 0:
            raise RuntimeError(f"axon_start_nrt_profile rc={rc}")
        try:
            yield
        finally:
            n = lib.axon_stop_nrt_profile(str(output_dir).encode())
            if n < 0:
                raise RuntimeError(f"axon_stop_nrt_profile rc={n}")

    return _hook


def _install_axon_hooks():
    if "antenv.axon_hooks" in sys.modules:
        return
    mod = types.ModuleType("antenv.axon_hooks")
    holder = [None]
    mod.set_axon_ntff_profile_hook = lambda h: holder.__setitem__(0, h)
    mod.get_axon_ntff_profile_hook = lambda: holder[0]
    sys.modules["antenv.axon_hooks"] = mod
    try:
        import antenv

        antenv.axon_hooks = mod
    except ImportError:
        pass
    if os.path.exists(_SO_PATH):
        mod.set_axon_ntff_profile_hook(_ntff_profile_via_ctypes(_SO_PATH))


_install_axon_hooks()

import concourse.bacc as bacc  # noqa: E402
import concourse.bass as bass  # noqa: E402
import concourse.mybir as mybir  # noqa: E402
import concourse.tile as tile  # noqa: E402
from concourse._compat import get_trn_type  # noqa: E402
from concourse.bass_utils import run_bass_kernel_spmd  # noqa: E402
from concourse.library_config import mlp as _mlp_lib  # noqa: E402

try:
    from concourse import bass_utils as _bu

    _bu.upload_artifacts = lambda tmpdir: tmpdir
except Exception:
    pass

# ---------------------------------------------------------------- wait legalization

_waitfix_counter = [0]


def _legalize_waits(nc):
    """walrus allows 1 sem wait on most instructions, 2 on EventSemaphore.
    Hoist excess waits onto standalone event-sem carriers on the same engine
    (the sequencer blocks on the carrier first; program order makes it
    equivalent)."""
    added = 0
    for bb in nc.main_func.blocks:
        insts = bb.instructions
        i = 0
        while i < len(insts):
            inst = insts[i]
            si = inst.sync_info
            if si is None or not si.on_wait:
                i += 1
                continue
            lim = 2 if type(inst).__name__ == "InstEventSemaphore" else 1
            waits = list(si.on_wait)
            if len(waits) <= lim:
                i += 1
                continue
            keep, excess = waits[:lim], waits[lim:]
            carriers = []
            for j in range(0, len(excess), 2):
                _waitfix_counter[0] += 1
                c = mybir.InstEventSemaphore(
                    name=f"waitfix_{_waitfix_counter[0]}", ins=[], outs=[]
                )
                c.engine = inst.engine
                c.sync_info = mybir.SyncInfo(on_wait=excess[j : j + 2], on_update=[])
                carriers.append(c)
            inst.sync_info = mybir.SyncInfo(on_wait=keep, on_update=list(si.on_update))
            for k, c in enumerate(carriers):
                insts.insert(i + k, c)
                nc.register_instruction(c, overwrite=True)
            added += len(carriers)
            i += len(carriers) + 1
    return added


# ---------------------------------------------------------------- constants

N_NODES = 10000
N_EDGES = 160000
HID = 512
HEADS = 4
N_GRAPHS = 8
NCORES = 8
SHARD = N_NODES // NCORES  # 1250
P = 128
NBLK = math.ceil(SHARD / P)  # 10
TW = 576  # h_ext table row width (f32); 576*4 % 256 == 0 for dma_gather
GSUB = 4  # chunks per dma_gather call (512 indices)
EPS_LN = 1e-5
LAYERS = [(4, 128), (4, 128), (1, 512)]  # (heads, ch/head)
F32 = mybir.dt.float32
F32R = mybir.dt.float32r
I16 = mybir.dt.int16


def _r(ap):
    return ap.bitcast(F32R)

_cache = {}


# ---------------------------------------------------------------- host prep


def _host_prep(edge_index, node_types, batch):
    src = np.concatenate([edge_index[0], np.arange(N_NODES, dtype=np.int64)])
    dst = np.concatenate([edge_index[1], np.arange(N_NODES, dtype=np.int64)])
    order = np.argsort(dst, kind="stable")
    src, dst = src[order], dst[order]
    # edges per (core, block): dst block = dst // 128 within shard
    # block boundaries in sorted dst via searchsorted
    starts = np.searchsorted(dst, np.arange(0, N_NODES + 1, P))  # 79 blocks of 128
    # but shard blocks: core c block b covers nodes [c*1250 + b*128, ...)
    block_lo = []
    block_hi = []
    for c in range(NCORES):
        base = c * SHARD
        for b in range(NBLK):
            lo = base + b * P
            hi = min(base + (b + 1) * P, base + SHARD)
            block_lo.append(lo)
            block_hi.append(hi)
    lo_idx = np.searchsorted(dst, np.array(block_lo))
    hi_idx = np.searchsorted(dst, np.array(block_hi))
    counts = (hi_idx - lo_idx).reshape(NCORES, NBLK)
    cpb = np.maximum(1, np.ceil(counts.max(axis=0) / P).astype(np.int64))  # [NBLK]
    totc = int(cpb.sum())

    # per-core tensors
    per_core = []
    # sub-gather segmentation (same for all cores): per block, chunks split
    # into calls of <= GSUB chunks
    subs = []  # list of (block, k0, kn, goff) with goff = global chunk offset
    goffs = np.concatenate([[0], np.cumsum(cpb)])
    for b in range(NBLK):
        k0 = 0
        while k0 < cpb[b]:
            kn = min(GSUB, int(cpb[b]) - k0)
            subs.append((b, k0, kn, int(goffs[b]) + k0))
            k0 += kn

    for c in range(NCORES):
        idx16 = np.zeros((16, totc * 8), np.int16)
        dstloc = np.full((P, totc), 255, np.uint8)
        dstrep = np.full((totc * P,), 255, np.uint8)
        for b in range(NBLK):
            i0, i1 = lo_idx[c * NBLK + b], hi_idx[c * NBLK + b]
            nsl = int(cpb[b]) * P
            s_pad = np.zeros(nsl, np.int64)
            d_pad = np.full(nsl, -1.0, np.float32)
            s_pad[: i1 - i0] = src[i0:i1]
            d_pad[: i1 - i0] = (dst[i0:i1] - block_lo[c * NBLK + b]).astype(np.float32)
            goff = int(goffs[b])
            dl = np.full((nsl,), 255, np.uint8)
            m2 = d_pad >= 0
            dl[m2] = d_pad[m2].astype(np.uint8)
            dstloc[:, goff : goff + int(cpb[b])] = dl.reshape(int(cpb[b]), P).T
            dmask = d_pad >= 0
            dstrep[goff * P : goff * P + nsl][dmask] = d_pad[dmask].astype(np.uint8)
            # idx16 layout per sub-gather: arr[kn,128] -> [16, kn*8]
            for bb2, k0, kn, go in subs:
                if bb2 != b:
                    continue
                arr = s_pad[k0 * P : (k0 + kn) * P].reshape(kn, 8, 16)
                lay = arr.transpose(2, 0, 1).reshape(16, kn * 8)
                idx16[:, go * 8 : (go + kn) * 8] = lay.astype(np.int16)
        idx16 = np.tile(idx16, (8, 1))

        base = c * SHARD
        ohb = np.zeros((P, NBLK * N_GRAPHS), np.float32)
        for b in range(NBLK):
            nb = min(P, SHARD - b * P)
            rows = batch[base + b * P : base + b * P + nb].astype(np.int64)
            ohb[np.arange(nb), b * N_GRAPHS + rows] = 1.0
        oh6 = np.zeros((6, SHARD), np.float32)
        oh6[node_types[base : base + SHARD].astype(np.int64), np.arange(SHARD)] = 1.0
        per_core.append(
            {
                "idx16": idx16,
                "dstloc": dstloc,
                "ohb": ohb,
                "oh6T": oh6,
                "dstrep": np.tile(dstrep[None, :], (P, 1)),
            }
        )
    meta = {
        "cpb": tuple(int(x) for x in cpb),
        "totc": totc,
        "subs": tuple(subs),
    }
    return meta, per_core


def _build_wext(W, a_src, a_dst, H, C):
    As = np.zeros((HID, H), np.float32)
    Ad = np.zeros((HID, H), np.float32)
    for h in range(H):
        As[h * C : (h + 1) * C, h] = a_src[h]
        Ad[h * C : (h + 1) * C, h] = a_dst[h]
    out = np.zeros((HID, TW), np.float32)
    out[:, :HID] = W
    out[:, HID : HID + H] = W @ As
    out[:, HID + H : HID + 2 * H] = W @ Ad
    return out


# ---------------------------------------------------------------- program


def _build_program(meta):
    cpb = meta["cpb"]
    totc = meta["totc"]
    subs = meta["subs"]
    cpbmax_sub = GSUB

    nc = bacc.Bacc(get_trn_type() or "TRN2")
    dt = F32

    # I/O
    xT_in = nc.dram_tensor("xT", [HID, SHARD], F32R, kind="ExternalInput")
    oh6T_in = nc.dram_tensor("oh6T", [6, SHARD], F32R, kind="ExternalInput")
    temb_in = nc.dram_tensor("temb", [6, HID], F32R, kind="ExternalInput")
    wext_in = nc.dram_tensor("wext", [3, HID, TW], F32R, kind="ExternalInput")
    brep_in = nc.dram_tensor("brep", [3, P, HID], dt, kind="ExternalInput")
    grep_in = nc.dram_tensor("grep", [3, P, HID], dt, kind="ExternalInput")
    lbrep_in = nc.dram_tensor("lbrep", [3, P, HID], dt, kind="ExternalInput")
    colidx_in = nc.dram_tensor("colidx", [P, P], mybir.dt.uint8, kind="ExternalInput")
    ident_in = nc.dram_tensor("ident", [P, P], dt, kind="ExternalInput")
    dstloc_in = nc.dram_tensor("dstloc", [P, totc], mybir.dt.uint8, kind="ExternalInput")
    dstrep_in = nc.dram_tensor("dstrep", [P, totc * P], mybir.dt.uint8, kind="ExternalInput")
    iotap_in = nc.dram_tensor("iotap", [P, 1], mybir.dt.uint8, kind="ExternalInput")
    idx16_in = nc.dram_tensor("idx16", [P, totc * 8], I16, kind="ExternalInput")
    ohb_in = nc.dram_tensor("ohb", [P, NBLK * N_GRAPHS], dt, kind="ExternalInput")
    invc_in = nc.dram_tensor("invc", [N_GRAPHS, 1], dt, kind="ExternalInput")
    out_nodes = nc.dram_tensor("out_nodes", [SHARD, HID], dt, kind="ExternalOutput")
    out_graph = nc.dram_tensor("out_graph", [N_GRAPHS, HID], dt, kind="ExternalOutput")

    # internal dram
    h_shard_d = nc.dram_tensor("h_shard_d", [SHARD, TW], F32R)
    h_full_d = nc.dram_tensor("h_full_d", [N_NODES, TW], F32R, addr_space="Shared")
    pool_in_d = nc.dram_tensor("pool_in_d", [N_GRAPHS, HID], dt)
    pool_out_d = nc.dram_tensor("pool_out_d", [N_GRAPHS, HID], dt, addr_space="Shared")

    nc.gpsimd.load_library(_mlp_lib)

    with tile.TileContext(nc) as tc:
        with (
            tc.tile_pool(name="const", bufs=1) as cst,
            tc.tile_pool(name="xt", bufs=1) as xtp,
            tc.tile_pool(name="xblk", bufs=1) as xbp,
            tc.tile_pool(name="wextp", bufs=1) as wxp,
            tc.tile_pool(name="g", bufs=4) as gp,
            tc.tile_pool(name="sel", bufs=2) as selp,
            tc.tile_pool(name="selt", bufs=2) as seltp,
            tc.tile_pool(name="wt", bufs=2) as wtp,
            tc.tile_pool(name="hstage", bufs=2) as hsp,
            tc.tile_pool(name="hz", bufs=3) as hzp,
            tc.tile_pool(name="pbig", bufs=2, space="PSUM") as pbig,
            tc.tile_pool(name="pselT", bufs=2, space="PSUM") as pselT,
            tc.tile_pool(name="ptiny", bufs=2, space="PSUM") as ptiny,
            tc.tile_pool(name="psacc", bufs=1, space="PSUM") as psacc,
            tc.tile_pool(name="ppool", bufs=1, space="PSUM") as ppoolp,
        ):
            V, A = nc.vector, nc.scalar
            Copy = mybir.ActivationFunctionType.Copy
            Ident = mybir.ActivationFunctionType.Identity
            Exp = mybir.ActivationFunctionType.Exp
            Ln = mybir.ActivationFunctionType.Ln
            Square = mybir.ActivationFunctionType.Square
            Relu = mybir.ActivationFunctionType.Relu

            # ---- consts
            colidx = cst.tile([P, P], mybir.dt.uint8)
            nc.sync.dma_start(colidx[:], colidx_in[:])
            ident = cst.tile([P, P], dt)
            nc.sync.dma_start(ident[:], ident_in[:])
            dstloc = cst.tile([P, totc], mybir.dt.uint8)
            nc.sync.dma_start(dstloc[:], dstloc_in[:])
            dstrep = cst.tile([P, totc * P], mybir.dt.uint8)
            nc.sync.dma_start(dstrep[:], dstrep_in[:])
            iota_u8 = cst.tile([P, 1], mybir.dt.uint8, tag="iota_u8")
            nc.sync.dma_start(iota_u8[:], iotap_in[:])
            idx16 = cst.tile([P, totc * 8], I16)
            nc.sync.dma_start(idx16[:], idx16_in[:])
            temb = cst.tile([6, HID], F32R)
            nc.sync.dma_start(temb[:], temb_in[:])
            oh6T = cst.tile([6, SHARD], F32R)
            nc.sync.dma_start(oh6T[:], oh6T_in[:])
            ohb = cst.tile([P, NBLK * N_GRAPHS], dt)
            nc.sync.dma_start(ohb[:], ohb_in[:])
            invc = cst.tile([N_GRAPHS, 1], dt)
            nc.sync.dma_start(invc[:], invc_in[:])
            breps, greps, lbreps = [], [], []
            for l in range(3):
                t = cst.tile([P, HID], dt, tag=f"brep{l}")
                nc.sync.dma_start(t[:], brep_in[l])
                breps.append(t)
                t = cst.tile([P, HID], dt, tag=f"grep{l}")
                nc.sync.dma_start(t[:], grep_in[l])
                greps.append(t)
                t = cst.tile([P, HID], dt, tag=f"lbrep{l}")
                nc.sync.dma_start(t[:], lbrep_in[l])
                lbreps.append(t)
            alphad = cst.tile([P, NBLK * 16], F32R)
            V.memset(alphad[:].bitcast(F32), 0.0)
            c_magic = cst.tile([P, 1], mybir.dt.int32, tag="c_magic")
            V.memset(c_magic[:], 0x5F3759DF)

            xt_a = xtp.tile([P, 4, SHARD], F32R, tag="xt_a")
            xt_b = xtp.tile([P, 4, SHARD], F32R, tag="xt_b")
            xblk_a = xbp.tile([P, NBLK, HID], dt, tag="xblk_a")
            xblk_b = xbp.tile([P, NBLK, HID], dt, tag="xblk_b")

            # ---- x0T = xT + type_emb.T @ oh6T
            nc.sync.dma_start(
                xt_a[:], xT_in[:].rearrange("(k p) n -> p k n", p=P)
            )
            ntiles = [(0, 512), (512, 512), (1024, SHARD - 1024)]
            for kc in range(4):
                for n0, nn in ntiles:
                    pt = pbig.tile([P, 512], dt, tag="big")
                    nc.tensor.matmul(
                        pt[:, :nn],
                        lhsT=temb[:, kc * P : (kc + 1) * P],
                        rhs=oh6T[:, n0 : n0 + nn],
                        start=True,
                        stop=True,
                    )
                    V.tensor_add(
                        xt_a[:, kc, n0 : n0 + nn],
                        xt_a[:, kc, n0 : n0 + nn],
                        pt[:, :nn],
                    )

            goffs = [0]
            for b in range(NBLK):
                goffs.append(goffs[-1] + cpb[b])

            ppool = ppoolp.tile([N_GRAPHS, HID], dt)

            xts = [xt_a, xt_b, xt_a]
            xbs = [xblk_a, xblk_b, xblk_a]  # output blocks per layer

            def emit_dense_block(lx, m, wx, xt_src):
                Hx = LAYERS[lx][0]
                nm = min(P, SHARD - m * P)
                ph = pbig.tile([P, 512], dt, tag="big")
                pa = ptiny.tile([P, 16 * GSUB], dt, tag="tiny")
                for kc in range(4):
                    lslice = xt_src[:, kc, m * P : m * P + nm]
                    nc.tensor.matmul(
                        ph[:nm, :],
                        lhsT=lslice,
                        rhs=wx[:, kc, 0:HID],
                        start=(kc == 0),
                        stop=(kc == 3),
                    )
                    nc.tensor.matmul(
                        pa[:nm, : 2 * Hx],
                        lhsT=lslice.bitcast(F32),
                        rhs=wx[:, kc, HID : HID + 2 * Hx].bitcast(F32),
                        start=(kc == 0),
                        stop=(kc == 3),
                    )
                hx = hsp.tile([P, TW], F32R, tag="hx")
                A.activation(hx[:nm, :HID], ph[:nm, :], Copy)
                A.activation(hx[:nm, HID : HID + 2 * Hx], pa[:nm, : 2 * Hx], Copy)
                V.tensor_copy(
                    alphad[:nm, m * 16 : m * 16 + Hx], pa[:nm, Hx : 2 * Hx]
                )
                nc.sync.dma_start(h_shard_d[m * P : m * P + nm, :], hx[:nm, :])

            def emit_ag():
                nc.gpsimd.collective_compute(
                    "AllGather",
                    mybir.AluOpType.bypass,
                    ins=[h_shard_d[:]],
                    outs=[h_full_d[:]],
                    replica_groups=[list(range(NCORES))],
                )

            # layer-0 dense up front
            wx = wxp.tile([P, 4, TW], F32R, tag="wext")
            nc.sync.dma_start(wx[:], wext_in[0].rearrange("(k p) w -> p k w", p=P))
            for m in range(NBLK):
                emit_dense_block(0, m, wx, xts[0])
            emit_ag()

            for l in range(3):
                H, C = LAYERS[l]
                xt_cur = xts[l]
                xb_out = xbs[l]
                xb_res = xbs[l - 1] if l >= 1 else None
                xt_nxt = xts[l + 1] if l < 2 else None

                # ---- gather phase
                for b in range(NBLK):
                    nb = min(P, SHARD - b * P)
                    pm = pbig.tile([P, 512], dt, tag="big")
                    ps = psacc.tile([P, 16], dt)
                    nchunks = cpb[b]
                    bsubs = [s for s in subs if s[0] == b]
                    for (bb, k0, kn, go) in bsubs:
                        G = gp.tile([P, cpbmax_sub, TW], F32R, tag="g")
                        nc.gpsimd.dma_gather(
                            G[:, :kn, :],
                            h_full_d[:],
                            idx16[:, go * 8 : (go + kn) * 8],
                            kn * P,
                            kn * P,
                            TW,
                            single_packet=False,
                        )
                        # group-batched selectors
                        sel8 = selp.tile([P, cpbmax_sub, P], F32R, tag="sel")
                        V.tensor_tensor(
                            out=sel8[:, :kn, :],
                            in0=dstloc[:, go : go + kn].to_broadcast([P, kn, P]),
                            in1=colidx[:]
                            .rearrange("p (a c) -> p a c", a=1)
                            .to_broadcast([P, kn, P]),
                            op=mybir.AluOpType.is_equal,
                        )
                        selT8 = seltp.tile([P, cpbmax_sub, P], F32R, tag="selT")
                        V.tensor_tensor(
                            out=selT8[:, :kn, :],
                            in0=iota_u8[:, 0:1].to_broadcast([P, kn, P]),
                            in1=dstrep[:, go * P : (go + kn) * P].rearrange(
                                "p (a c) -> p a c", c=P
                            ),
                            op=mybir.AluOpType.is_equal,
                        )
                        # alpha_dst per edge: kn small matmuls into one psum bank
                        pad8 = ptiny.tile([P, 16 * cpbmax_sub], dt, tag="tiny")
                        for kk in range(kn):
                            nc.tensor.matmul(
                                pad8[:, kk * 16 : kk * 16 + 16],
                                lhsT=selT8[:, kk, :],
                                rhs=alphad[:, b * 16 : b * 16 + 16],
                                start=True,
                                stop=True,
                            )
                        # batched scores: es = alpha_src + alpha_dst; w = exp(leaky(es))
                        es = wtp.tile([P, 4 * cpbmax_sub], dt, tag="es")
                        V.tensor_tensor(
                            out=es[:, : kn * H].rearrange("p (a c) -> p a c", c=H),
                            in0=G[:, :kn, HID : HID + H],
                            in1=pad8[:, : kn * 16]
                            .rearrange("p (a c) -> p a c", c=16)[:, :, :H],
                            op=mybir.AluOpType.add,
                        )
                        lk = wtp.tile([P, 4 * cpbmax_sub], dt, tag="lk")
                        V.tensor_scalar(
                            out=lk[:, : kn * H],
                            in0=es[:, : kn * H],
                            scalar1=0.2,
                            scalar2=None,
                            op0=mybir.AluOpType.mult,
                        )
                        V.tensor_tensor(
                            out=lk[:, : kn * H],
                            in0=es[:, : kn * H],
                            in1=lk[:, : kn * H],
                            op=mybir.AluOpType.max,
                        )
                        A.activation(
                            G[:, :kn, HID + 2 * H : HID + 3 * H],
                            lk[:, : kn * H].rearrange("p (a h) -> p a h", h=H),
                            Exp,
                        )
                        # multiply w into G in place (rounding to f32r for PE),
                        # and stash w into the pad columns of each row
                        V.tensor_tensor(
                            out=G[:, :kn, :HID].rearrange(
                                "p a (h c) -> p a h c", h=H
                            ),
                            in0=G[:, :kn, :HID].rearrange(
                                "p a (h c) -> p a h c", h=H
                            ),
                            in1=G[:, :kn, HID + 2 * H : HID + 3 * H].to_broadcast(
                                [P, kn, H, C]
                            ),
                            op=mybir.AluOpType.mult,
                        )
                        for kk in range(kn):
                            k = k0 + kk
                            nc.tensor.matmul(
                                pm[:],
                                lhsT=sel8[:, kk, :],
                                rhs=G[:, kk, :HID],
                                start=(k == 0),
                                stop=(k == nchunks - 1),
                            )
                            nc.tensor.matmul(
                                ps[:, :16],
                                lhsT=sel8[:, kk, :],
                                rhs=G[:, kk, HID + 2 * H : HID + 2 * H + 16],
                                start=(k == 0),
                                stop=(k == nchunks - 1),
                            )
                    # ---- block epilogue
                    sS = wtp.tile([P, 8], dt, tag="sS")
                    A.activation(sS[:, :H], ps[:, :H], Copy, bias=1e-16)
                    rs = wtp.tile([P, 8], dt, tag="rs")
                    V.reciprocal(rs[:, :H], sS[:, :H])
                    zt = hzp.tile([P, HID], dt, tag="hz")
                    z = zt
                    for h in range(H):
                        A.activation(
                            z[:, h * C : (h + 1) * C],
                            pm[:, h * C : (h + 1) * C],
                            Copy,
                            scale=rs[:, h : h + 1],
                        )
                    V.tensor_add(z[:, :HID], z[:, :HID], breps[l][:])
                    if l < 2:
                        z2 = hzp.tile([P, HID], dt, tag="hz")
                        A.activation(z2[:, :HID], z[:, :HID], Relu)
                        z = z2
                    # LayerNorm
                    sumz = wtp.tile([P, 8], dt, tag="sumz")
                    V.reduce_sum(sumz[:, 0:1], z[:, :HID], axis=mybir.AxisListType.X)
                    mu = wtp.tile([P, 8], dt, tag="mu")
                    V.tensor_scalar(
                        out=mu[:, 0:1], in0=sumz[:, 0:1], scalar1=1.0 / HID,
                        scalar2=None, op0=mybir.AluOpType.mult,
                    )
                    y0 = hzp.tile([P, HID], dt, tag="hz")
                    s2 = wtp.tile([P, 8], dt, tag="s2")
                    A.activation(y0[:, :HID], z[:, :HID], Square, accum_out=s2[:, 0:1])
                    stat = wtp.tile([P, 8], dt, tag="stat")
                    V.tensor_scalar(
                        out=stat[:, 0:1], in0=s2[:, 0:1], scalar1=1.0 / HID,
                        scalar2=None, op0=mybir.AluOpType.mult,
                    )
                    V.tensor_tensor(
                        out=stat[:, 1:2], in0=mu[:, 0:1], in1=mu[:, 0:1],
                        op=mybir.AluOpType.mult,
                    )
                    V.tensor_tensor(
                        out=stat[:, 2:3], in0=stat[:, 0:1], in1=stat[:, 1:2],
                        op=mybir.AluOpType.subtract,
                    )
                    V.tensor_scalar(
                        out=stat[:, 2:3], in0=stat[:, 2:3], scalar1=EPS_LN,
                        scalar2=None, op0=mybir.AluOpType.add,
                    )
                    # rstd = rsqrt(var) via bit-trick + 3 Newton steps (DVE only;
                    # ACT sqrt has poor precision and lives in another table set)
                    vi = stat[:, 2:3].bitcast(mybir.dt.int32)
                    si = stat[:, 7:8].bitcast(mybir.dt.int32)
                    V.tensor_scalar(
                        out=si, in0=vi, scalar1=1, scalar2=None,
                        op0=mybir.AluOpType.logical_shift_right,
                    )
                    V.tensor_tensor(
                        out=si, in0=c_magic[:, 0:1], in1=si,
                        op=mybir.AluOpType.subtract,
                    )
                    yn = stat[:, 7:8]
                    tn = stat[:, 3:4]
                    for _ in range(3):
                        V.tensor_tensor(out=tn, in0=yn, in1=yn, op=mybir.AluOpType.mult)
                        V.tensor_tensor(out=tn, in0=tn, in1=stat[:, 2:3], op=mybir.AluOpType.mult)
                        V.tensor_scalar(out=tn, in0=tn, scalar1=-0.5, scalar2=None, op0=mybir.AluOpType.mult)
                        V.tensor_scalar(out=tn, in0=tn, scalar1=1.5, scalar2=None, op0=mybir.AluOpType.add)
                        V.tensor_tensor(out=yn, in0=yn, in1=tn, op=mybir.AluOpType.mult)
                    V.tensor_copy(stat[:, 4:5], yn)
                    V.tensor_tensor(
                        out=stat[:, 5:6], in0=mu[:, 0:1], in1=stat[:, 4:5],
                        op=mybir.AluOpType.mult,
                    )
                    V.tensor_scalar(
                        out=stat[:, 6:7], in0=stat[:, 5:6], scalar1=-1.0,
                        scalar2=None, op0=mybir.AluOpType.mult,
                    )
                    A.activation(
                        y0[:, :HID], z[:, :HID], Ident, scale=stat[:, 4:5], bias=stat[:, 6:7]
                    )
                    V.tensor_tensor(
                        out=y0[:, :HID], in0=y0[:, :HID], in1=greps[l][:], op=mybir.AluOpType.mult
                    )
                    ob = xb_out[:, b, :]
                    V.tensor_tensor(
                        out=ob, in0=y0[:, :HID], in1=lbreps[l][:], op=mybir.AluOpType.add
                    )
                    if l >= 1:
                        V.tensor_tensor(
                            out=ob, in0=ob, in1=xb_res[:, b, :],
                            op=mybir.AluOpType.add,
                        )
                    if l < 2:
                        for kc in range(4):
                            pT2 = pselT.tile([P, P], dt, tag="selT")
                            nc.tensor.transpose(
                                pT2[:], xb_out[:, b, kc * P : (kc + 1) * P], ident[:]
                            )
                            A.activation(
                                xt_nxt[:, kc, b * P : b * P + nb],
                                pT2[:, :nb],
                                Copy,
                            )
                    else:
                        nc.sync.dma_start(
                            out_nodes[b * P : b * P + nb, :], ob[:nb]
                        )
                        nc.tensor.matmul(
                            ppool[:],
                            lhsT=ohb[:, b * N_GRAPHS : (b + 1) * N_GRAPHS],
                            rhs=ob,
                            start=(b == 0),
                            stop=(b == NBLK - 1),
                        )

                if l < 2:
                    wx = wxp.tile([P, 4, TW], F32R, tag="wext")
                    nc.sync.dma_start(
                        wx[:], wext_in[l + 1].rearrange("(k p) w -> p k w", p=P)
                    )
                    for m in range(NBLK):
                        emit_dense_block(l + 1, m, wx, xt_nxt)
                    emit_ag()

            # ---- graph pooling
            pc = cst.tile([N_GRAPHS, HID], dt, tag="poolc")
            A.activation(pc[:], ppool[:], Copy)
            nc.sync.dma_start(pool_in_d[:], pc[:])
            nc.gpsimd.collective_compute(
                "AllReduce",
                mybir.AluOpType.add,
                ins=[pool_in_d[:]],
                outs=[pool_out_d[:]],
                replica_groups=[list(range(NCORES))],
            )
            pg = cst.tile([N_GRAPHS, HID], dt, tag="poolg")
            nc.sync.dma_start(pg[:], pool_out_d[:])
            V.tensor_scalar(
                out=pg[:], in0=pg[:], scalar1=invc[:, 0:1], scalar2=None,
                op0=mybir.AluOpType.mult,
            )
            nc.sync.dma_start(out_graph[:], pg[:])

    nc.compile()
    _legalize_waits(nc)
    return nc


# ---------------------------------------------------------------- runner


def _prepare(inputs):
    edge_index = np.asarray(inputs["edge_index"])
    node_types = np.asarray(inputs["node_types"])
    batch = np.asarray(inputs["batch"])
    meta, per_core = _host_prep(edge_index, node_types, batch)

    key = (meta["cpb"], meta["totc"])
    if key not in _cache:
        _cache.clear()
        _cache[key] = _build_program(meta)
    nc = _cache[key]

    x = np.ascontiguousarray(np.asarray(inputs["x"], np.float32))
    wexts = np.stack(
        [
            _build_wext(
                np.asarray(inputs[f"gat{l}_W"], np.float32),
                np.asarray(inputs[f"gat{l}_asrc"], np.float32),
                np.asarray(inputs[f"gat{l}_adst"], np.float32),
                *LAYERS[l],
            )
            for l in range(3)
        ]
    )
    brep = np.stack(
        [np.tile(np.asarray(inputs[f"gat{l}_b"], np.float32), (P, 1)) for l in range(3)]
    )
    ln_g = np.asarray(inputs["ln_g"], np.float32)
    ln_b = np.asarray(inputs["ln_b"], np.float32)
    grep = np.stack([np.tile(ln_g[l], (P, 1)) for l in range(3)])
    lbrep = np.stack([np.tile(ln_b[l], (P, 1)) for l in range(3)])
    colidx = np.tile(np.arange(P, dtype=np.uint8), (P, 1))
    ident = np.eye(P, dtype=np.float32)
    cnt = np.bincount(batch.astype(np.int64), minlength=N_GRAPHS).astype(np.float32)
    invc = (1.0 / np.maximum(cnt, 1.0)).reshape(N_GRAPHS, 1)

    in_maps = []
    for c in range(NCORES):
        base = c * SHARD
        pc = per_core[c]
        in_maps.append(
            {
                "xT": np.ascontiguousarray(x[base : base + SHARD].T),
                "oh6T": pc["oh6T"],
                "temb": np.asarray(inputs["type_emb"], np.float32),
                "wext": wexts,
                "brep": brep,
                "grep": grep,
                "lbrep": lbrep,
                "colidx": colidx,
                "ident": ident,
                "dstloc": pc["dstloc"],
                "dstrep": pc["dstrep"],
                "iotap": np.arange(P, dtype=np.uint8).reshape(P, 1),
                "idx16": pc["idx16"],
                "ohb": pc["ohb"],
                "invc": invc,
            }
        )
    return nc, in_maps


def _run(inputs, trace=False, tmpdir=None):
    nc, in_maps = _prepare(inputs)
    res = run_bass_kernel_spmd(
        nc, in_maps, list(range(NCORES)), trace=trace, tmpdir=tmpdir
    )
    node_emb = np.concatenate(
        [res.results[c]["out_nodes"] for c in range(NCORES)], axis=0
    )
    graph_emb = res.results[0]["out_graph"]
    return (node_emb, graph_emb), res


def kernel(**inputs):
    out, _ = _run(inputs)
    return out
